# revision 46
# baseline (speedup 1.0000x reference)
"""Trainium2 Bass kernel for DigitConvolutionalModel (8-core data parallel).

Computation: x(B,784) -> 3x3 valid conv on 28x28 -> flatten(676)
             -> FC(100)+ReLU -> FC(10), B = 65536.

Algebraic restructure (host side, exact): the conv is linear, so conv and
fc1 fold into one 784->100 matrix W1eff (accumulated in float64). The
device kernel is then just two matmul layers per 512-sample tile:
  h = relu(x @ W1eff + b1);  y = h @ fc2_w.T + b2.

Numerics: the matmul datapath runs in fp16 (inputs rounded once on the
host). Measured end-to-end scale-relative absmax error vs the fp32
reference is ~4.5e-4; fp16 streams the PE at 1 col/cycle (fp32 runs at
~1/4 rate) and halves the HBM traffic, which is the kernel's bottleneck.

Per-core layout (B_shard=8192 = 16 tiles x 512):
  x is pre-transposed on the host to feature-major tiles so the matmul
  contraction (features) lands on SBUF partitions with no on-device
  transposes. Features 0..767 form 6 chunks of 128 partitions (full DMA
  port utilization, fully contiguous 0.77 MB loads alternating across
  the two HWDGE rings); the 16 remainder features for all 16 tiles are
  packed into one [128, 2048] tile at 32-aligned partition groups (PE
  row-group granularity) and applied with per-group w1r replicas.
  Outputs accumulate in SBUF and leave in tapered writebacks so only
  one 20 KB write remains after the final tile.
"""

import numpy as np

import concourse.bass as bass
import concourse.mybir as mybir
import concourse.tile as tile
from concourse.bass_utils import run_bass_kernel_spmd
from concourse.vector_clock import ScopedClock

N_CORES = 8
B_TOTAL = 65536
B_SHARD = B_TOTAL // N_CORES  # 8192
BT = 512  # batch tile (one PSUM bank of fp32)
N_TILES = B_SHARD // BT  # 16
FC = 6  # full 128-partition feature chunks (6*128 = 768)
F_REM = 784 - FC * 128  # 16 remainder features
H1 = 100
H2 = 10

_f32 = mybir.dt.float32
_f32r = mybir.dt.float32r
_f16 = mybir.dt.float16


class SplitDrainTileContext(tile.TileContext):
    """TileContext whose tail drain carries at most one sync wait.

    The pinned walrus rejects instructions with >2 sync waits
    ("Too many sync wait commands" in setupSyncWait); the stock tail
    drain accumulates one wait per active proc. Emit one drain per
    wait instead — consecutive drains on the sync engine are
    semantically equivalent to one drain carrying all the waits.
    """

    def _drain_and_barrier(self, tick_clock, wait_clock):
        nc = self.nc
        # Cheap tail: the stock version runs two full EVSEM butterflies
        # (~13us measured). Instead: gpsimd waits on the whole vector
        # clock (all tracked incs have landed), every engine drains its
        # own DGE queues, gpsimd clears the sem ranges, and one
        # sequencer-level sem-only barrier closes the kernel.
        drain_inst = nc.gpsimd.drain()
        wait_clock.add_sem_waits(
            drain_inst.ins, ScopedClock({None: tick_clock.global_clock})
        )
        raw = drain_inst.ins
        si = raw.sync_info
        if si is not None and si.on_wait and len(si.on_wait) > 1:
            waits = list(si.on_wait)
            si.on_wait = waits[:1]
            raw.sync_info = si
            for w in waits[1:]:
                extra = nc.gpsimd.drain()
                extra.ins.sync_info = mybir.SyncInfo(on_wait=[w], on_update=[])
        for eng in (nc.sync, nc.scalar, nc.vector, nc.tensor):
            eng.drain()

        # No tail barrier: gpsimd's global-clock waits above guarantee all
        # tracked sem incs (incl. DMA completions) have landed before the
        # clears, and NRT serializes re-executions on all-engine completion.
        assert self.sems is not None
        popped = nc._tile_sem_poison_stack.pop()
        assert popped is self._sem_poison
        nc.clear_and_free_semaphores(list(self.sems.allocated().values()))


def _split_sync_waits(nc: bass.Bass, limit: int = 1) -> None:
    """Walrus-compat post-pass: the pinned walrus rejects instructions
    carrying more than ~2 sync waits. Hoist excess waits onto NoOp
    instructions inserted just before the offending instruction on the
    same engine — semantically identical (waits run in stream order)."""
    n = 0
    for fn in nc.m.functions:
        for bb in fn.blocks:
            out = []
            changed = False
            for inst in bb.instructions:
                si = inst.sync_info
                if si is not None and si.on_wait and len(si.on_wait) > limit:
                    waits = list(si.on_wait)
                    for i in range(0, len(waits) - limit, limit):
                        nop = mybir.InstNoOp(
                            name=f"swsplit-{n}",
                            ins=[],
                            outs=[],
                            sync_info=mybir.SyncInfo(
                                on_wait=waits[i : i + limit], on_update=[]
                            ),
                        )
                        nop.engine = inst.engine
                        out.append(nop)
                        n += 1
                    si.on_wait = waits[len(waits) - limit :]
                    inst.sync_info = si
                    changed = True
                out.append(inst)
            if changed:
                bb.instructions = out


def _build_nc(x_bufs: int = 8) -> bass.Bass:
    nc = bass.Bass(monotonic_sem_count=0)
    xm = nc.dram_tensor("xm", [N_TILES, 128, FC, BT], _f16, kind="ExternalInput")
    # remainder features packed 4 tile-groups x 16 features into 128
    # partitions at 32-aligned offsets (PE row-group granularity)
    xr = nc.dram_tensor("xr", [128, 4 * BT], _f16, kind="ExternalInput")
    w1m = nc.dram_tensor("w1m", [128, FC * H1], _f16, kind="ExternalInput")
    # w1r replicated at partition offsets 0/32/64/96
    w1r = nc.dram_tensor("w1r", [128, H1], _f16, kind="ExternalInput")
    b1 = nc.dram_tensor("b1", [H1, 1], _f32, kind="ExternalInput")
    w2 = nc.dram_tensor("w2", [H1, H2], _f16, kind="ExternalInput")
    b2 = nc.dram_tensor("b2", [H2, 1], _f32, kind="ExternalInput")
    y = nc.dram_tensor("y", [H2, N_TILES * BT], _f32, kind="ExternalOutput")

    with SplitDrainTileContext(nc) as tc:
        with (
            tc.tile_pool(name="consts", bufs=1) as cpool,
            tc.tile_pool(name="xp", bufs=x_bufs) as xpool,
            tc.tile_pool(name="hp", bufs=4) as hpool,
            tc.tile_pool(name="psh", bufs=4, space="PSUM") as psh,
            tc.tile_pool(name="pso", bufs=3, space="PSUM") as pso,
            tc.tile_pool(name="wps", bufs=1, space="PSUM") as wpool,
        ):
            # big consts ride the HWDGE rings ahead of the x stream;
            # tiny ones go to the otherwise-idle SWDGE queue.
            # w1m chunk 0 loads separately so the first matmul can start
            # as soon as the first half of x0 lands.
            w1m_sb = cpool.tile([128, FC * H1], _f16, tag="w1m")
            nc.sync.dma_start(out=w1m_sb[:, :H1], in_=w1m[:, :H1])
            nc.sync.dma_start(out=w1m_sb[:, H1:], in_=w1m[:, H1:])
            xr_sb = cpool.tile([128, 4 * BT], _f16, tag="xr")
            w1r_sb = cpool.tile([128, H1], _f16, tag="w1r")
            b1_sb = cpool.tile([H1, 1], _f32, tag="b1")
            w2_sb = cpool.tile([H1, H2], _f16, tag="w2")
            b2_sb = cpool.tile([H2, 1], _f32, tag="b2")
            # outputs accumulate here; written back in two half DMAs
            o_sb = cpool.tile([H2, N_TILES * BT], _f32, tag="o")

            # PE pre-warm: HAM keeps the PE at 1.2 GHz until ~3.4us of
            # sustained activity. Run dummy matmuls on a memset tile while
            # the first x tiles stream in, sized to end as x0 lands, so the
            # real matmul stream starts at 2.4 GHz.
            warm_sb = cpool.tile([128, 64], _f16, tag="warm")
            nc.vector.memset(warm_sb[:], 0)
            warm_ps = wpool.tile([64, 64], _f32, tag="wps")
            for _ in range(170):
                nc.tensor.matmul(
                    warm_ps[:], warm_sb[:, :64], warm_sb[:, :64], start=True, stop=True
                )

            # process tiles in pairs sharing each stationary operand so the
            # PE sees LDW, MM, MM chains instead of LDW, MM, LDW, MM
            for t in range(N_TILES):
                x_sb = xpool.tile([128, FC * BT], _f16, tag="x")
                # alternate HWDGE rings (SP / ACT) for the bulk loads
                dma_eng = nc.sync if t % 2 == 0 else nc.scalar
                src = xm[t].rearrange("p c b -> p (c b)")
                if t < 2:
                    # split the first loads so the PE can start sooner
                    hw = FC * BT // 2
                    dma_eng.dma_start(out=x_sb[:, :hw], in_=src[:, :hw])
                    dma_eng.dma_start(out=x_sb[:, hw:], in_=src[:, hw:])
                else:
                    dma_eng.dma_start(out=x_sb[:], in_=src)
                if t == 1:
                    # small consts ride the scalar ring behind x1 (slack
                    # there) — keeping SWDGE unused saves 8 DMASW sems and
                    # their per-execution NRT end-protocol cost
                    nc.scalar.dma_start(out=xr_sb[:], in_=xr[:])
                    nc.scalar.dma_start(out=w1r_sb[:], in_=w1r[:])
                    nc.scalar.dma_start(out=b1_sb[:], in_=b1[:])
                    nc.scalar.dma_start(out=w2_sb[:], in_=w2[:])
                    nc.scalar.dma_start(out=b2_sb[:], in_=b2[:])
                if t % 2 == 0:
                    x_pair = x_sb
                    continue

                g = t // 4
                phs = [psh.tile([H1, BT], _f32, tag="ph", name="ph") for _ in range(2)]
                for c in range(FC + 1):
                    for k, (xs_, tt) in enumerate(((x_pair, t - 1), (x_sb, t))):
                        if c < FC:
                            nc.tensor.matmul(
                                phs[k][:],
                                w1m_sb[:, c * H1 : (c + 1) * H1],
                                xs_[:, c * BT : (c + 1) * BT],
                                start=(c == 0),
                                stop=False,
                            )
                        else:
                            q = tt % 4
                            nc.tensor.matmul(
                                phs[k][:],
                                w1r_sb[32 * g : 32 * g + F_REM, :],
                                xr_sb[32 * g : 32 * g + F_REM, q * BT : (q + 1) * BT],
                                start=False,
                                stop=True,
                                tile_position=(96, 0) if g == 3 else None,
                            )

                # relu(ph + b1) on DVE — ACT stays a pure DMA-issue engine so
                # its HWDGE ring never stalls behind compute
                hs = [hpool.tile([H1, BT], _f16, tag="h", name="h") for _ in range(2)]
                for k in range(2):
                    nc.vector.tensor_scalar(
                        hs[k][:],
                        phs[k][:],
                        b1_sb[:, 0:1],
                        0.0,
                        mybir.AluOpType.add,
                        mybir.AluOpType.max,
                    )

                pos = [pso.tile([H2, BT], _f32, tag="po", name="po") for _ in range(2)]
                for k in range(2):
                    nc.tensor.matmul(
                        pos[k][:],
                        w2_sb[:],
                        hs[k][:],
                        start=True,
                        stop=True,
                    )

                for k in range(2):
                    tt = t - 1 + k
                    nc.vector.tensor_scalar_add(
                        o_sb[:, tt * BT : (tt + 1) * BT], pos[k][:], b2_sb[:, 0:1]
                    )
                # tapered writeback: big chunks leave mid-kernel, the
                # final write after the last tile is only one tile (20 KB)
                flush = {7: [(0, 8)], 11: [(8, 12)], 13: [(12, 14)],
                         15: [(14, 15), (15, 16)]}
                for i, (a, b) in enumerate(flush.get(t, [])):
                    eng = nc.sync if (t + i) % 2 == 1 else nc.scalar
                    eng.dma_start(
                        out=y[:, a * BT : b * BT], in_=o_sb[:, a * BT : b * BT]
                    )

    _split_sync_waits(nc)
    return nc


def _fold_conv_fc1(conv_w: np.ndarray, fc1_w: np.ndarray) -> np.ndarray:
    """Fold the 3x3 valid conv into fc1: W1eff[784, 100] such that
    h = x @ W1eff  ==  fc1( flatten(conv(x)) ).  Accumulated in float64."""
    F = fc1_w.astype(np.float64).T.reshape(26, 26, H1)
    W = np.zeros((28, 28, H1), np.float64)
    cw = conv_w.astype(np.float64)
    for di in range(3):
        for dj in range(3):
            W[di : di + 26, dj : dj + 26, :] += cw[di, dj] * F
    return W.reshape(784, H1).astype(np.float32)


def _make_in_maps(x, conv_w, fc1_w, fc1_b, fc2_w, fc2_b):
    w1eff = _fold_conv_fc1(conv_w, fc1_w)
    w1m = np.ascontiguousarray(
        w1eff[: FC * 128]
        .astype(np.float16)
        .reshape(FC, 128, H1)
        .transpose(1, 0, 2)
        .reshape(128, FC * H1)
    )
    w1r = np.zeros((128, H1), np.float16)
    for g in range(4):
        w1r[32 * g : 32 * g + F_REM] = w1eff[FC * 128 :].astype(np.float16)
    b1 = np.ascontiguousarray(fc1_b.reshape(H1, 1))
    w2 = np.ascontiguousarray(fc2_w.T.astype(np.float16))
    b2 = np.ascontiguousarray(fc2_b.reshape(H2, 1))

    in_maps = []
    for s in range(N_CORES):
        xs = x[s * B_SHARD : (s + 1) * B_SHARD].reshape(N_TILES, BT, 784)
        xm = np.ascontiguousarray(
            xs[:, :, : FC * 128]
            .astype(np.float16)
            .reshape(N_TILES, BT, FC, 128)
            .transpose(0, 3, 2, 1)
        )
        xr_flat = xs.reshape(B_SHARD, 784)[:, FC * 128 :].astype(np.float16)
        xr = np.zeros((128, 4 * BT), np.float16)
        for t in range(N_TILES):
            g, q = t // 4, t % 4
            xr[32 * g : 32 * g + F_REM, q * BT : (q + 1) * BT] = xr_flat[
                t * BT : (t + 1) * BT
            ].T
        in_maps.append(
            {"xm": xm, "xr": xr, "w1m": w1m, "w1r": w1r, "b1": b1, "w2": w2, "b2": b2}
        )
    return in_maps


def _gather(results) -> np.ndarray:
    out = np.empty((B_TOTAL, H2), np.float32)
    for s in range(N_CORES):
        ys = results[s]["y"]  # [H2, B_SHARD]
        out[s * B_SHARD : (s + 1) * B_SHARD] = ys.T
    return out


def kernel_run(inputs: dict, trace: bool = False):
    """Run the kernel; returns (full output (65536,10) f32, BassKernelResults)."""
    x = np.ascontiguousarray(np.asarray(inputs["x"], dtype=np.float32))
    assert x.shape == (B_TOTAL, 784), x.shape
    in_maps = _make_in_maps(
        x,
        np.asarray(inputs["conv_w"], np.float32),
        np.asarray(inputs["fc1_w"], np.float32),
        np.asarray(inputs["fc1_b"], np.float32),
        np.asarray(inputs["fc2_w"], np.float32),
        np.asarray(inputs["fc2_b"], np.float32),
    )
    nc = _build_nc()
    res = run_bass_kernel_spmd(nc, in_maps, core_ids=list(range(N_CORES)), trace=trace)
    return _gather(res.results), res


def kernel(**inputs) -> np.ndarray:
    out, _ = kernel_run(inputs)
    return out


# revision 47
# speedup vs baseline: 1.0615x; 1.0615x over previous
"""Trainium2 Bass kernel for DigitConvolutionalModel (8-core data parallel).

Computation: x(B,784) -> 3x3 valid conv on 28x28 -> flatten(676)
             -> FC(100)+ReLU -> FC(10), B = 65536.

Algebraic restructure (host side, exact): the conv is linear, so conv and
fc1 fold into one 784->100 matrix W1eff (accumulated in float64). The
device kernel is then just two matmul layers per 512-sample tile:
  h = relu(x @ W1eff + b1);  y = h @ fc2_w.T + b2.

Numerics: the matmul datapath runs in fp16 (inputs rounded once on the
host). Measured end-to-end scale-relative absmax error vs the fp32
reference is ~4.5e-4; fp16 streams the PE at 1 col/cycle (fp32 runs at
~1/4 rate) and halves the HBM traffic, which is the kernel's bottleneck.

Per-core layout (B_shard=8192 = 16 tiles x 512):
  x is pre-transposed on the host to feature-major tiles so the matmul
  contraction (features) lands on SBUF partitions with no on-device
  transposes. Features 0..767 form 6 chunks of 128 partitions (full DMA
  port utilization, fully contiguous 0.77 MB loads alternating across
  the two HWDGE rings); the 16 remainder features for all 16 tiles are
  packed into one [128, 2048] tile at 32-aligned partition groups (PE
  row-group granularity) and applied with per-group w1r replicas.
  Outputs accumulate in SBUF and leave in tapered writebacks so only
  one 20 KB write remains after the final tile.
"""

import numpy as np

import concourse.bass as bass
import concourse.mybir as mybir
import concourse.tile as tile
from concourse.bass_utils import run_bass_kernel_spmd
from concourse.vector_clock import ScopedClock

N_CORES = 8
B_TOTAL = 65536
B_SHARD = B_TOTAL // N_CORES  # 8192
BT = 512  # batch tile (one PSUM bank of fp32)
N_TILES = B_SHARD // BT  # 16
FC = 6  # full 128-partition feature chunks (6*128 = 768)
F_REM = 784 - FC * 128  # 16 remainder features
H1 = 100
H2 = 10

_f32 = mybir.dt.float32
_f32r = mybir.dt.float32r
_f16 = mybir.dt.float16


class SplitDrainTileContext(tile.TileContext):
    """TileContext whose tail drain carries at most one sync wait.

    The pinned walrus rejects instructions with >2 sync waits
    ("Too many sync wait commands" in setupSyncWait); the stock tail
    drain accumulates one wait per active proc. Emit one drain per
    wait instead — consecutive drains on the sync engine are
    semantically equivalent to one drain carrying all the waits.
    """

    def _drain_and_barrier(self, tick_clock, wait_clock):
        nc = self.nc
        # Cheap tail: the stock version runs two full EVSEM butterflies
        # (~13us measured). Instead: gpsimd waits on the whole vector
        # clock (all tracked incs have landed), every engine drains its
        # own DGE queues, gpsimd clears the sem ranges, and one
        # sequencer-level sem-only barrier closes the kernel.
        drain_inst = nc.gpsimd.drain()
        wait_clock.add_sem_waits(
            drain_inst.ins, ScopedClock({None: tick_clock.global_clock})
        )
        raw = drain_inst.ins
        si = raw.sync_info
        if si is not None and si.on_wait and len(si.on_wait) > 1:
            waits = list(si.on_wait)
            si.on_wait = waits[:1]
            raw.sync_info = si
            for w in waits[1:]:
                extra = nc.gpsimd.drain()
                extra.ins.sync_info = mybir.SyncInfo(on_wait=[w], on_update=[])
        for eng in (nc.sync, nc.scalar, nc.vector, nc.tensor):
            eng.drain()

        # No tail barrier: gpsimd's global-clock waits above guarantee all
        # tracked sem incs (incl. DMA completions) have landed before the
        # clears, and NRT serializes re-executions on all-engine completion.
        assert self.sems is not None
        popped = nc._tile_sem_poison_stack.pop()
        assert popped is self._sem_poison
        nc.clear_and_free_semaphores(list(self.sems.allocated().values()))


def _split_sync_waits(nc: bass.Bass, limit: int = 1) -> None:
    """Walrus-compat post-pass: the pinned walrus rejects instructions
    carrying more than ~2 sync waits. Hoist excess waits onto NoOp
    instructions inserted just before the offending instruction on the
    same engine — semantically identical (waits run in stream order)."""
    n = 0
    for fn in nc.m.functions:
        for bb in fn.blocks:
            out = []
            changed = False
            for inst in bb.instructions:
                si = inst.sync_info
                if si is not None and si.on_wait and len(si.on_wait) > limit:
                    waits = list(si.on_wait)
                    for i in range(0, len(waits) - limit, limit):
                        nop = mybir.InstNoOp(
                            name=f"swsplit-{n}",
                            ins=[],
                            outs=[],
                            sync_info=mybir.SyncInfo(
                                on_wait=waits[i : i + limit], on_update=[]
                            ),
                        )
                        nop.engine = inst.engine
                        out.append(nop)
                        n += 1
                    si.on_wait = waits[len(waits) - limit :]
                    inst.sync_info = si
                    changed = True
                out.append(inst)
            if changed:
                bb.instructions = out


def _build_nc(x_bufs: int = 8) -> bass.Bass:
    nc = bass.Bass(monotonic_sem_count=0)
    xm = nc.dram_tensor("xm", [N_TILES, 128, FC, BT], _f16, kind="ExternalInput")
    # remainder features packed 4 tile-groups x 16 features into 128
    # partitions at 32-aligned offsets (PE row-group granularity)
    xr = nc.dram_tensor("xr", [128, 4 * BT], _f16, kind="ExternalInput")
    w1m = nc.dram_tensor("w1m", [128, FC * H1], _f16, kind="ExternalInput")
    # w1r replicated at partition offsets 0/32/64/96
    w1r = nc.dram_tensor("w1r", [128, H1], _f16, kind="ExternalInput")
    b1 = nc.dram_tensor("b1", [H1, 1], _f32, kind="ExternalInput")
    w2 = nc.dram_tensor("w2", [H1, H2], _f16, kind="ExternalInput")
    b2 = nc.dram_tensor("b2", [H2, 1], _f32, kind="ExternalInput")
    y = nc.dram_tensor("y", [H2, N_TILES * BT], _f32, kind="ExternalOutput")

    with SplitDrainTileContext(nc) as tc:
        with (
            tc.tile_pool(name="consts", bufs=1) as cpool,
            tc.tile_pool(name="xp", bufs=x_bufs) as xpool,
            tc.tile_pool(name="hp", bufs=4) as hpool,
            tc.tile_pool(name="psh", bufs=4, space="PSUM") as psh,
            tc.tile_pool(name="pso", bufs=3, space="PSUM") as pso,
            tc.tile_pool(name="wps", bufs=1, space="PSUM") as wpool,
        ):
            # big consts ride the HWDGE rings ahead of the x stream;
            # tiny ones go to the otherwise-idle SWDGE queue.
            # w1m chunk 0 loads separately so the first matmul can start
            # as soon as the first half of x0 lands.
            w1m_sb = cpool.tile([128, FC * H1], _f16, tag="w1m")
            nc.sync.dma_start(out=w1m_sb[:, :H1], in_=w1m[:, :H1])
            nc.sync.dma_start(out=w1m_sb[:, H1:], in_=w1m[:, H1:])
            xr_sb = cpool.tile([128, 4 * BT], _f16, tag="xr")
            w1r_sb = cpool.tile([128, H1], _f16, tag="w1r")
            b1_sb = cpool.tile([H1, 1], _f32, tag="b1")
            w2_sb = cpool.tile([H1, H2], _f16, tag="w2")
            b2_sb = cpool.tile([H2, 1], _f32, tag="b2")
            # outputs accumulate here; written back in two half DMAs
            o_sb = cpool.tile([H2, N_TILES * BT], _f32, tag="o")

            # PE pre-warm: HAM keeps the PE at 1.2 GHz until ~3.4us of
            # sustained activity. Run dummy matmuls on a memset tile while
            # the first x tiles stream in, sized to end as x0 lands, so the
            # real matmul stream starts at 2.4 GHz.
            warm_sb = cpool.tile([128, 64], _f16, tag="warm")
            nc.vector.memset(warm_sb[:], 0)
            warm_ps = wpool.tile([64, 64], _f32, tag="wps")
            for _ in range(220):
                nc.tensor.matmul(
                    warm_ps[:], warm_sb[:, :64], warm_sb[:, :64], start=True, stop=True
                )

            # process tiles in pairs sharing each stationary operand so the
            # PE sees LDW, MM, MM chains instead of LDW, MM, LDW, MM
            for t in range(N_TILES):
                x_sb = xpool.tile([128, FC * BT], _f16, tag="x")
                # alternate HWDGE rings (SP / ACT) for the bulk loads
                dma_eng = nc.sync if t % 2 == 0 else nc.scalar
                src = xm[t].rearrange("p c b -> p (c b)")
                if t < 2:
                    # split the first loads so the PE can start sooner
                    hw = FC * BT // 2
                    dma_eng.dma_start(out=x_sb[:, :hw], in_=src[:, :hw])
                    dma_eng.dma_start(out=x_sb[:, hw:], in_=src[:, hw:])
                else:
                    dma_eng.dma_start(out=x_sb[:], in_=src)
                if t == 1:
                    # small consts ride the scalar ring behind x1 (slack
                    # there) — keeping SWDGE unused saves 8 DMASW sems and
                    # their per-execution NRT end-protocol cost
                    nc.scalar.dma_start(out=xr_sb[:], in_=xr[:])
                    nc.scalar.dma_start(out=w1r_sb[:], in_=w1r[:])
                    nc.scalar.dma_start(out=b1_sb[:], in_=b1[:])
                    nc.scalar.dma_start(out=w2_sb[:], in_=w2[:])
                    nc.scalar.dma_start(out=b2_sb[:], in_=b2[:])
                if t % 2 == 0:
                    x_pair = x_sb
                    continue

                g = t // 4
                phs = [psh.tile([H1, BT], _f32, tag="ph", name="ph") for _ in range(2)]
                for c in range(FC + 1):
                    for k, (xs_, tt) in enumerate(((x_pair, t - 1), (x_sb, t))):
                        if c < FC:
                            nc.tensor.matmul(
                                phs[k][:],
                                w1m_sb[:, c * H1 : (c + 1) * H1],
                                xs_[:, c * BT : (c + 1) * BT],
                                start=(c == 0),
                                stop=False,
                            )
                        else:
                            q = tt % 4
                            nc.tensor.matmul(
                                phs[k][:],
                                w1r_sb[32 * g : 32 * g + F_REM, :],
                                xr_sb[32 * g : 32 * g + F_REM, q * BT : (q + 1) * BT],
                                start=False,
                                stop=True,
                                tile_position=(96, 0) if g == 3 else None,
                            )

                # relu(ph + b1) on DVE — ACT stays a pure DMA-issue engine so
                # its HWDGE ring never stalls behind compute
                hs = [hpool.tile([H1, BT], _f16, tag="h", name="h") for _ in range(2)]
                for k in range(2):
                    nc.vector.tensor_scalar(
                        hs[k][:],
                        phs[k][:],
                        b1_sb[:, 0:1],
                        0.0,
                        mybir.AluOpType.add,
                        mybir.AluOpType.max,
                    )

                pos = [pso.tile([H2, BT], _f32, tag="po", name="po") for _ in range(2)]
                for k in range(2):
                    nc.tensor.matmul(
                        pos[k][:],
                        w2_sb[:],
                        hs[k][:],
                        start=True,
                        stop=True,
                    )

                for k in range(2):
                    tt = t - 1 + k
                    nc.vector.tensor_scalar_add(
                        o_sb[:, tt * BT : (tt + 1) * BT], pos[k][:], b2_sb[:, 0:1]
                    )
                # tapered writeback: big chunks leave mid-kernel, the
                # final write after the last tile is only one tile (20 KB)
                flush = {7: [(0, 8)], 11: [(8, 12)], 13: [(12, 14)],
                         15: [(14, 15), (15, 16)]}
                for i, (a, b) in enumerate(flush.get(t, [])):
                    eng = nc.sync if (t + i) % 2 == 1 else nc.scalar
                    eng.dma_start(
                        out=y[:, a * BT : b * BT], in_=o_sb[:, a * BT : b * BT]
                    )

    _split_sync_waits(nc)
    return nc


def _fold_conv_fc1(conv_w: np.ndarray, fc1_w: np.ndarray) -> np.ndarray:
    """Fold the 3x3 valid conv into fc1: W1eff[784, 100] such that
    h = x @ W1eff  ==  fc1( flatten(conv(x)) ).  Accumulated in float64."""
    F = fc1_w.astype(np.float64).T.reshape(26, 26, H1)
    W = np.zeros((28, 28, H1), np.float64)
    cw = conv_w.astype(np.float64)
    for di in range(3):
        for dj in range(3):
            W[di : di + 26, dj : dj + 26, :] += cw[di, dj] * F
    return W.reshape(784, H1).astype(np.float32)


def _make_in_maps(x, conv_w, fc1_w, fc1_b, fc2_w, fc2_b):
    w1eff = _fold_conv_fc1(conv_w, fc1_w)
    w1m = np.ascontiguousarray(
        w1eff[: FC * 128]
        .astype(np.float16)
        .reshape(FC, 128, H1)
        .transpose(1, 0, 2)
        .reshape(128, FC * H1)
    )
    w1r = np.zeros((128, H1), np.float16)
    for g in range(4):
        w1r[32 * g : 32 * g + F_REM] = w1eff[FC * 128 :].astype(np.float16)
    b1 = np.ascontiguousarray(fc1_b.reshape(H1, 1))
    w2 = np.ascontiguousarray(fc2_w.T.astype(np.float16))
    b2 = np.ascontiguousarray(fc2_b.reshape(H2, 1))

    in_maps = []
    for s in range(N_CORES):
        xs = x[s * B_SHARD : (s + 1) * B_SHARD].reshape(N_TILES, BT, 784)
        xm = np.ascontiguousarray(
            xs[:, :, : FC * 128]
            .astype(np.float16)
            .reshape(N_TILES, BT, FC, 128)
            .transpose(0, 3, 2, 1)
        )
        xr_flat = xs.reshape(B_SHARD, 784)[:, FC * 128 :].astype(np.float16)
        xr = np.zeros((128, 4 * BT), np.float16)
        for t in range(N_TILES):
            g, q = t // 4, t % 4
            xr[32 * g : 32 * g + F_REM, q * BT : (q + 1) * BT] = xr_flat[
                t * BT : (t + 1) * BT
            ].T
        in_maps.append(
            {"xm": xm, "xr": xr, "w1m": w1m, "w1r": w1r, "b1": b1, "w2": w2, "b2": b2}
        )
    return in_maps


def _gather(results) -> np.ndarray:
    out = np.empty((B_TOTAL, H2), np.float32)
    for s in range(N_CORES):
        ys = results[s]["y"]  # [H2, B_SHARD]
        out[s * B_SHARD : (s + 1) * B_SHARD] = ys.T
    return out


def kernel_run(inputs: dict, trace: bool = False):
    """Run the kernel; returns (full output (65536,10) f32, BassKernelResults)."""
    x = np.ascontiguousarray(np.asarray(inputs["x"], dtype=np.float32))
    assert x.shape == (B_TOTAL, 784), x.shape
    in_maps = _make_in_maps(
        x,
        np.asarray(inputs["conv_w"], np.float32),
        np.asarray(inputs["fc1_w"], np.float32),
        np.asarray(inputs["fc1_b"], np.float32),
        np.asarray(inputs["fc2_w"], np.float32),
        np.asarray(inputs["fc2_b"], np.float32),
    )
    nc = _build_nc()
    res = run_bass_kernel_spmd(nc, in_maps, core_ids=list(range(N_CORES)), trace=trace)
    return _gather(res.results), res


def kernel(**inputs) -> np.ndarray:
    out, _ = kernel_run(inputs)
    return out
